# revision 23
# baseline (speedup 1.0000x reference)
"""AnimeStyleAttention distributed Bass kernel for 8 TRN2 NeuronCores.

Full module: y = (softmax(q k^T / 8) v  *  gate(style)) @ Wo + bo
  with q/k/v = x @ W{q,k,v} + b,  gate = sigmoid(gelu(style@Ws1+bs1)@Ws2+bs2)

Sharding: core c -> (batch b = c//2, head-group g = c%2).  Each core handles
one batch element and 4 of the 8 heads (a 256-channel slice of the QKV space).
Per-batch partial outputs (out_heads/den) @ (Wo_rows * gate) are summed
pairwise on the host (bo added by even cores only; host sums in f32).

Design (v2):
  * The ScalarE exp stream is the hard floor (~143us: 128 ACTIVATEs of
    [128,2,512] psum->sbuf at ~1.11us).  Everything else is arranged so that
    stream starts early and never stalls:
    - one ACT table set for the whole kernel (gelu/sigmoid via tanh
      identities, tanh lives in the exp set); table preloaded at t~0
    - inputs packed on the host into few DMAs (w4 = Wq|Wk|Wv|Ws2 in one
      transfer, all f32 bias vectors in another) so the DMA semaphore pool
      doesn't serialize the x transposes
    - flat software pipeline over 128 global kt-steps: scores(k)/exp(k)
      issue 2 slots ahead of attnv(k-2), so chunk boundaries never gap the
      exp stream; projections / out-proj / style are filler between steps
    - chunk drain is split: two DVE casts free the attention psum banks
      (~1.3us), then reciprocal+normalize run later entirely off SBUF
  * PSUM budget (8 banks): scores 2x[128,2,512] (4) + attn out 2x[65,512]
    (2) + shared proj/rcb/y pool 2x[128,512] (2).

Layouts on chip (per core, N=2048 tokens, D=512, CH=256, Dh=64):
  xT   [128, 4, 2048] fp16   x^T: in-channel on partitions
  qT/kT[128, 2, 2048] fp16   (2 head-pairs; partitions 0:64=lo head, 64:128=hi)
  vv   [128, 16, 4, 65] fp16 natural [token, ch]; 65th col = ones (denominator)
  scores^T = k @ q^T : [k-tok partitions, q-tok free], row-packed pairs of
  K=64 matmuls (tile_position from base partition).  exp on ScalarE
  (no max subtraction: |scores| <~ 2), psum->sbuf fp16.
  attn.v + denominator accumulate in PSUM via the ones column (M=65).
  Drain: psum -> sbuf fp16 cast; 1/den via K=1 ones-matmul broadcast of the
  fp16 den row + one wide DVE reciprocal_approx_fast; zT = out * recip.
  y = zT^T @ (Wo*gate) + bo -> DRAM f16 (host accumulates f32).
"""

from contextlib import ExitStack

import numpy as np

import concourse.bacc as bacc
import concourse.bass as bass
import concourse.tile as tile
from concourse import mybir

P = 128
N = 2048          # tokens (one batch element per core)
D = 512           # model dim
CH = 256          # this core's qkv channel slice (4 heads x 64)
NKT = N // P      # 16 token tiles
QC = 512          # q-chunk width
NQC = N // QC     # 4
NC = 8            # chunks = (pr, qc)
LAG = 2           # attnv trails scores by this many kt-steps
F16 = mybir.dt.float16
F32 = mybir.dt.float32
AF = mybir.ActivationFunctionType
ALU = mybir.AluOpType

GELU_C = 0.7978845608028654   # sqrt(2/pi)
GELU_A = 0.044715


def build_program() -> bass.Bass:
    nc = bacc.Bacc()

    x_d = nc.declare_dram_parameter("x", [N, D], F16, isOutput=False)
    # packed: Wq | Wk | Wv | Ws2 along columns
    w4_d = nc.declare_dram_parameter("w4", [D, 4 * CH], F16, isOutput=False)
    wo_d = nc.declare_dram_parameter("wo", [CH, D], F16, isOutput=False)
    ws1_d = nc.declare_dram_parameter("ws1", [D, D], F16, isOutput=False)
    st_d = nc.declare_dram_parameter("style", [D], F16, isOutput=False)
    # packed f32 smalls: bq | bk | bs1 | bs2 | bo  (256+256+512+256+512)
    sm_d = nc.declare_dram_parameter("sm", [1792], F32, isOutput=False)
    bv_d = nc.declare_dram_parameter("bv", [CH], F32, isOutput=False)
    out_d = nc.declare_dram_parameter("out", [N, D], F16, isOutput=True)

    with ExitStack() as ctx:
        tc = ctx.enter_context(tile.TileContext(nc))
        const = ctx.enter_context(tc.tile_pool(name="const", bufs=1))

        # ---- warmup activation first on the ACT queue: pulls the single
        # exp_and_others table load to t~0, overlapping the input DMAs ----
        warm_in = const.tile([1, 2], F32)
        nc.vector.memset(warm_in, 0.0)
        warm_out = const.tile([1, 2], F16)
        nc.scalar.activation(warm_out, warm_in, AF.Exp)

        # ---- input DMAs (few, fat) ----
        xTs = [const.tile([P, N], F16, name=f"xT{i}") for i in range(4)]
        nc.sync.dma_start_transpose(xTs[0], x_d[:, 0:P])
        nc.scalar.dma_start_transpose(xTs[1], x_d[:, P : 2 * P])
        nc.sync.dma_start_transpose(xTs[2], x_d[:, 2 * P : 3 * P])
        nc.scalar.dma_start_transpose(xTs[3], x_d[:, 3 * P : 4 * P])
        w4 = const.tile([P, 4, 4 * CH], F16)
        nc.gpsimd.dma_start(w4, w4_d.rearrange("(k p) m -> p k m", p=P))
        wq = w4[:, :, 0:CH]
        wk = w4[:, :, CH : 2 * CH]
        wv = w4[:, :, 2 * CH : 3 * CH]
        ws2 = w4[:, :, 3 * CH : 4 * CH]
        smT = const.tile([P, 14], F32)
        nc.gpsimd.dma_start(smT, sm_d.rearrange("(c p) -> p c", p=P))
        bqT = smT[:, 0:2]
        bkT = smT[:, 2:4]
        bs1T = smT[:, 4:8]
        bs2T = smT[:, 8:10]
        ws1 = const.tile([P, 4, D], F16)
        nc.gpsimd.dma_start(ws1, ws1_d.rearrange("(k p) m -> p k m", p=P))
        stT = const.tile([P, 4], F16)
        nc.gpsimd.dma_start(stT, st_d.rearrange("(c p) -> p c", p=P))
        bvb = const.tile([P, CH], F32)
        nc.sync.dma_start(bvb, bv_d.rearrange("(o c) -> o c", o=1).to_broadcast((P, CH)))
        wo = const.tile([P, 2, D], F16)
        nc.gpsimd.dma_start(wo, wo_d.rearrange("(k p) m -> p k m", p=P))
        bob = const.tile([P, D], F32)
        nc.sync.dma_start(
            bob,
            sm_d[1280:1792].rearrange("(o c) -> o c", o=1).to_broadcast((P, D)),
        )

        ones_row = const.tile([1, 64], F16)
        nc.vector.memset(ones_row, 1.0)
        wrm2 = const.tile([P, 512], F16)
        nc.vector.memset(wrm2, 0.02)

        qT = const.tile([P, 2, N], F16)
        kT = const.tile([P, 2, N], F16)
        vv = const.tile([P, NKT, 4, 65], F16)  # per head: 64 v cols + ones col
        nc.vector.memset(vv[:, :, :, 64:65], 1.0)
        zT = const.tile([P, 2, N], F16)
        gate = const.tile([P, 2], F32)
        wog = const.tile([P, 2, D], F16)       # Wo * gate (rows scaled)

        with (
            tc.tile_pool(name="scp", bufs=2, space="PSUM") as scp,
            tc.tile_pool(name="outp", bufs=2, space="PSUM") as outp,
            tc.tile_pool(name="mps", bufs=2, space="PSUM") as mps,
            tc.tile_pool(name="eep", bufs=8) as eep,
            tc.tile_pool(name="osb", bufs=4) as osb,
            tc.tile_pool(name="dra", bufs=4) as dra,
            tc.tile_pool(name="ysb", bufs=3) as ysb,
        ):
            def emit_qk(dst, w, bias, m, qc):
                s_ = slice(qc * QC, (qc + 1) * QC)
                ps = mps.tile([P, QC], F32, tag="m")
                for k in range(4):
                    nc.tensor.matmul(
                        ps,
                        lhsT=w[:, k, m * P : (m + 1) * P],
                        rhs=xTs[k][:, s_],
                        start=(k == 0),
                        stop=(k == 3),
                    )
                nc.vector.tensor_scalar_add(dst[:, m, s_], ps, bias[:, m : m + 1])

            def emit_v(tt):
                psv = mps.tile([P, CH], F32, tag="m")
                for k in range(4):
                    nc.tensor.matmul(
                        psv,
                        lhsT=xTs[k][:, tt * P : (tt + 1) * P],
                        rhs=wv[:, k, :],
                        start=(k == 0),
                        stop=(k == 3),
                    )
                nc.vector.tensor_add(
                    vv[:, tt, :, 0:64],
                    psv.rearrange("p (h w) -> p h w", w=64),
                    bvb.rearrange("p (h w) -> p h w", w=64),
                )

            style_xg = [None]

            def emit_style_mm():
                hps = mps.tile([P, 4], F32, tag="m")
                for m in range(4):
                    for k in range(4):
                        nc.tensor.matmul(
                            hps[:, m : m + 1],
                            lhsT=ws1[:, k, m * P : (m + 1) * P],
                            rhs=stT[:, k : k + 1],
                            start=(k == 0),
                            stop=(k == 3),
                        )
                xg = dra.tile([P, 4], F32, tag="stx")
                nc.vector.tensor_add(xg, hps, bs1T)
                style_xg[0] = xg

            def emit_style_act():
                # gelu(x) ~= 0.5*x*(1+tanh(c*(x+a*x^3)));  sigmoid via tanh
                xg = style_xg[0]
                x2 = dra.tile([P, 4], F32, tag="st")
                nc.vector.tensor_mul(x2, xg, xg)
                x3 = dra.tile([P, 4], F32, tag="st")
                nc.vector.tensor_mul(x3, x2, xg)
                u0 = dra.tile([P, 4], F32, tag="st")
                nc.vector.tensor_scalar_mul(u0, x3, GELU_A)
                u = dra.tile([P, 4], F32, tag="st")
                nc.vector.tensor_add(u, u0, xg)
                th = dra.tile([P, 4], F32, tag="st")
                nc.scalar.activation(th, u, AF.Tanh, scale=GELU_C)
                g1 = dra.tile([P, 4], F32, tag="st")
                nc.vector.tensor_mul(g1, xg, th)
                g2 = dra.tile([P, 4], F32, tag="st")
                nc.vector.tensor_add(g2, g1, xg)
                hT = dra.tile([P, 4], F16, tag="st16")
                nc.vector.tensor_scalar_mul(hT, g2, 0.5)
                g_ps = mps.tile([P, 2], F32, tag="m")
                for m in range(2):
                    for k in range(4):
                        nc.tensor.matmul(
                            g_ps[:, m : m + 1],
                            lhsT=ws2[:, k, m * P : (m + 1) * P],
                            rhs=hT[:, k : k + 1],
                            start=(k == 0),
                            stop=(k == 3),
                        )
                pre = dra.tile([P, 2], F32, tag="st")
                nc.vector.tensor_add(pre, g_ps, bs2T)
                th2 = dra.tile([P, 2], F32, tag="st")
                nc.scalar.activation(th2, pre, AF.Tanh, scale=0.5)
                nc.vector.tensor_scalar(gate, th2, 0.5, 0.5, op0=ALU.mult, op1=ALU.add)
                for m in range(2):
                    nc.vector.tensor_scalar_mul(
                        wog[:, m, :], wo[:, m, :], gate[:, m : m + 1]
                    )

            def emit_y_tt(tt):
                ps = mps.tile([P, D], F32, tag="m")
                for m in range(2):
                    nc.tensor.matmul(
                        ps,
                        lhsT=zT[:, m, tt * P : (tt + 1) * P],
                        rhs=wog[:, m, :],
                        start=(m == 0),
                        stop=(m == 1),
                    )
                y = ysb.tile([P, D], F16, tag="ys")
                with nc.allow_low_precision(reason="fp16 output vs f32 ref"):
                    nc.vector.tensor_add(y, ps, bob)
                eng = nc.sync if tt % 2 == 0 else nc.gpsimd
                eng.dma_start(out_d[tt * P : (tt + 1) * P, :], y)

            # ---- PE warmup: ~5us of FAT dummy matmuls (K=M=128 so the HAM
            # activity monitor sees the array busy) while the x transposes
            # land; HAM un-throttles (1.2->2.4GHz) after ~3.4us sustained ----
            def emit_warm(n, width):
                for _ in range(n):
                    wps = mps.tile([P, width], F32, tag="m")
                    nc.tensor.matmul(
                        wps, lhsT=wrm2[:, 0:P], rhs=wrm2[:, 0:width],
                        start=True, stop=True,
                    )

            emit_warm(12, 512)

            # ---- pre-loop: minimum to unlock scores of chunk (pr0, qc0) ----
            emit_qk(kT, wk, bkT, 0, 0)
            emit_qk(qT, wq, bqT, 0, 0)

            # ---- filler schedule keyed by global kt-step ----
            fill = {}

            def add(k, fn):
                fill.setdefault(k, []).append(fn)

            add(0, lambda: emit_qk(kT, wk, bkT, 0, 1))
            add(5, lambda: emit_qk(kT, wk, bkT, 0, 2))
            add(9, lambda: emit_qk(kT, wk, bkT, 0, 3))
            for tt in range(NKT):           # v(tt) before attnv(tt) at k=tt+2
                add(tt + 1, lambda tt=tt: emit_v(tt))
            add(13, emit_style_mm)
            add(15, lambda: emit_qk(qT, wq, bqT, 0, 1))
            add(19, emit_style_act)
            add(18, lambda: emit_qk(kT, wk, bkT, 1, 0))
            add(24, lambda: emit_qk(kT, wk, bkT, 1, 1))
            add(20, lambda: emit_qk(qT, wq, bqT, 0, 2))
            add(30, lambda: emit_qk(kT, wk, bkT, 1, 2))
            add(36, lambda: emit_qk(kT, wk, bkT, 1, 3))
            add(40, lambda: emit_qk(qT, wq, bqT, 0, 3))
            add(52, lambda: emit_qk(qT, wq, bqT, 1, 0))
            add(68, lambda: emit_qk(qT, wq, bqT, 1, 1))
            add(84, lambda: emit_qk(qT, wq, bqT, 1, 2))
            add(100, lambda: emit_qk(qT, wq, bqT, 1, 3))
            # keep-warm: one fat dummy matmul per late step so HAM doesn't
            # re-throttle before the tail (drain+out-proj of the last chunk)
            for k in range(90, 130, 2):
                add(k, lambda: emit_warm(1, 256))
            # y(qc): zT for (1,qc) ready after drainB of chunk 4+qc (k=16c+19)
            for qc in range(3):
                for tt in range(4):
                    add(16 * (5 + qc) + 8 + 2 * tt,
                        lambda tt=tt, qc=qc: emit_y_tt(4 * qc + tt))

            chunks = [(pr, qc) for pr in range(2) for qc in range(NQC)]
            outs = {}    # c -> (out_lo, out_hi) psum
            outsb = {}   # c -> (sb_lo, sb_hi) sbuf f16 copies

            def drain_cast(c):
                # out psum stays until drain_norm (scheduler hoists the rcb
                # matmuls ahead of the waiting attnv, so no deadlock)
                outsb[c] = outs.pop(c)

            def drain_norm(c, final=False):
                # 1/den broadcast + normalize (v1-proven op shapes: custom-DVE
                # reciprocal must NOT have a partition-shifted input)
                pr, qc = chunks[c]
                s = slice(qc * QC, (qc + 1) * QC)
                out_lo, out_hi = outsb.pop(c)
                d16l = dra.tile([1, QC], F16, tag="r16")
                d16h = dra.tile([1, QC], F16, tag="r16")
                with nc.allow_low_precision(reason="fp16 denom O(1e3)"):
                    nc.vector.tensor_copy(d16l, out_lo[64:65, :])
                    nc.vector.tensor_copy(d16h, out_hi[64:65, :])
                rcb = mps.tile([P, QC], F32, tag="m")
                nc.tensor.matmul(
                    rcb[0:64, :], lhsT=ones_row, rhs=d16l,
                    start=True, stop=True, tile_position=(0, 0),
                )
                nc.tensor.matmul(
                    rcb[64:128, :], lhsT=ones_row, rhs=d16h,
                    start=True, stop=True, tile_position=(0, 64),
                )
                rcb32 = dra.tile([P, QC], F32, tag="rw")
                nc.vector.reciprocal_approx_fast(out=rcb32, in_=rcb)
                if final:
                    # 128-col pieces so each token tile's out-proj starts as
                    # soon as its zT slice is ready (shortens the tail)
                    for t4 in range(4):
                        cs = slice(t4 * P, (t4 + 1) * P)
                        so = slice(qc * QC + t4 * P, qc * QC + (t4 + 1) * P)
                        nc.vector.tensor_mul(
                            zT[0:64, pr, so], out_lo[0:64, cs], rcb32[0:64, cs]
                        )
                        nc.vector.tensor_mul(
                            zT[64:128, pr, so], out_hi[0:64, cs], rcb32[64:128, cs]
                        )
                        emit_y_tt(qc * 4 + t4)
                else:
                    nc.vector.tensor_mul(
                        zT[0:64, pr, s], out_lo[0:64, :], rcb32[0:64, :]
                    )
                    nc.vector.tensor_mul(
                        zT[64:128, pr, s], out_hi[0:64, :], rcb32[64:128, :]
                    )

            # ---- flat pipeline over 128 kt-steps (+LAG attnv slots) ----
            attnv_args = []
            for k in range(NKT * NC + LAG):
                if k < NKT * NC:
                    c, kt = divmod(k, NKT)
                    pr, qc = chunks[c]
                    if kt == 0:
                        outs[c] = (
                            outp.tile([65, QC], F32, tag="o", name=f"out_lo{c}"),
                            outp.tile([65, QC], F32, tag="o", name=f"out_hi{c}"),
                        )
                    s = slice(qc * QC, (qc + 1) * QC)
                    ks = slice(kt * P, (kt + 1) * P)
                    sc = scp.tile([P, 2, QC], F32, tag="sc")
                    nc.tensor.matmul(
                        sc[:, 0, :], lhsT=kT[0:64, pr, ks], rhs=qT[0:64, pr, s],
                        start=True, stop=True,
                    )
                    nc.tensor.matmul(
                        sc[:, 1, :], lhsT=kT[64:128, pr, ks], rhs=qT[64:128, pr, s],
                        start=True, stop=True,
                    )
                    ee = eep.tile([P, 2, QC], F16, tag="e")
                    nc.scalar.activation(ee, sc, AF.Exp)
                    attnv_args.append((ee[:, 0, :], ee[:, 1, :]))
                # previous chunk's cast-drain goes before this chunk's first
                # attnv (frees the out psum slots with just 2 DVE casts)
                j = k - LAG
                if j >= 0:
                    cj, kj = divmod(j, NKT)
                    if kj == 0 and cj >= 1:
                        drain_cast(cj - 1)
                for fn in fill.get(k, ()):
                    fn()
                if j >= 0:
                    cj, kj = divmod(j, NKT)
                    prj, _ = chunks[cj]
                    av_lo, av_hi = attnv_args[j]
                    nc.tensor.matmul(
                        outs[cj][0],
                        lhsT=vv[:, kj, 2 * prj, :],
                        rhs=av_lo,
                        start=(kj == 0), stop=(kj == NKT - 1),
                    )
                    nc.tensor.matmul(
                        outs[cj][1],
                        lhsT=vv[:, kj, 2 * prj + 1, :],
                        rhs=av_hi,
                        start=(kj == 0), stop=(kj == NKT - 1),
                    )
                    if kj == 1 and cj >= 1:
                        drain_norm(cj - 1)

            drain_cast(NC - 1)
            drain_norm(NC - 1, final=True)

    nc.finalize()
    return nc


_NC_CACHE = None


def _get_program() -> bass.Bass:
    global _NC_CACHE
    if _NC_CACHE is None:
        _NC_CACHE = build_program()
    return _NC_CACHE


def make_in_maps(inputs: dict) -> list[dict]:
    f16 = np.float16
    f32 = np.float32
    scale = 1.0 / 8.0  # 1/sqrt(head_dim), folded into Wq/bq
    x, style = inputs["x"], inputs["style"]
    in_maps = []
    for c in range(8):
        b, g = divmod(c, 2)
        ch = slice(CH * g, CH * (g + 1))
        w4 = np.concatenate(
            [
                inputs["Wq"][:, ch] * scale,
                inputs["Wk"][:, ch],
                inputs["Wv"][:, ch],
                inputs["Ws2"][:, ch],
            ],
            axis=1,
        )
        bo = inputs["bo"] if g == 0 else np.zeros_like(inputs["bo"])
        sm = np.concatenate(
            [
                inputs["bq"][ch] * scale,
                inputs["bk"][ch],
                inputs["bs1"],
                inputs["bs2"][ch],
                bo,
            ]
        )
        in_maps.append(
            {
                "x": np.ascontiguousarray(x[b]).astype(f16),
                "style": np.ascontiguousarray(style[b]).astype(f16),
                "w4": np.ascontiguousarray(w4).astype(f16),
                "wo": np.ascontiguousarray(inputs["Wo"][ch, :]).astype(f16),
                "ws1": np.ascontiguousarray(inputs["Ws1"]).astype(f16),
                "sm": np.ascontiguousarray(sm).astype(f32),
                "bv": np.ascontiguousarray(inputs["bv"][ch]).astype(f32),
            }
        )
    return in_maps


def kernel(**inputs) -> np.ndarray:
    from concourse.bass_utils import run_bass_kernel_spmd

    in_maps = make_in_maps(inputs)
    res = run_bass_kernel_spmd(_get_program(), in_maps, list(range(8))).results
    y = np.stack(
        [
            res[2 * b]["out"].astype(np.float32)
            + res[2 * b + 1]["out"].astype(np.float32)
            for b in range(4)
        ]
    )
    return y.astype(np.float32)


# revision 24
# speedup vs baseline: 1.1924x; 1.1924x over previous
"""AnimeStyleAttention distributed Bass kernel for 8 TRN2 NeuronCores.

Full module: y = (softmax(q k^T / 8) v  *  gate(style)) @ Wo + bo
  with q/k/v = x @ W{q,k,v} + b,  gate = sigmoid(gelu(style@Ws1+bs1)@Ws2+bs2)

Sharding: core c -> (batch b = c//2, head-group g = c%2).  Each core handles
one batch element and 4 of the 8 heads (a 256-channel slice of the QKV space).
Per-batch partial outputs (out_heads/den) @ (Wo_rows * gate) are summed
pairwise on the host (bo added by even cores only; host sums in f32).

Design (v2):
  * The ScalarE exp stream is the hard floor (~143us: 128 ACTIVATEs of
    [128,2,512] psum->sbuf at ~1.11us).  Everything else is arranged so that
    stream starts early and never stalls:
    - one ACT table set for the whole kernel (gelu/sigmoid via tanh
      identities, tanh lives in the exp set); table preloaded at t~0
    - inputs packed on the host into few DMAs (w4 = Wq|Wk|Wv|Ws2 in one
      transfer, all f32 bias vectors in another) so the DMA semaphore pool
      doesn't serialize the x transposes
    - flat software pipeline over 128 global kt-steps: scores(k)/exp(k)
      issue 2 slots ahead of attnv(k-2), so chunk boundaries never gap the
      exp stream; projections / out-proj / style are filler between steps
    - chunk drain is split: two DVE casts free the attention psum banks
      (~1.3us), then reciprocal+normalize run later entirely off SBUF
  * PSUM budget (8 banks): scores 2x[128,2,512] (4) + attn out 2x[65,512]
    (2) + shared proj/rcb/y pool 2x[128,512] (2).

Layouts on chip (per core, N=2048 tokens, D=512, CH=256, Dh=64):
  xT   [128, 4, 2048] fp16   x^T: in-channel on partitions
  qT/kT[128, 2, 2048] fp16   (2 head-pairs; partitions 0:64=lo head, 64:128=hi)
  vv   [128, 16, 4, 65] fp16 natural [token, ch]; 65th col = ones (denominator)
  scores^T = k @ q^T : [k-tok partitions, q-tok free], row-packed pairs of
  K=64 matmuls (tile_position from base partition).  exp on ScalarE
  (no max subtraction: |scores| <~ 2), psum->sbuf fp16.
  attn.v + denominator accumulate in PSUM via the ones column (M=65).
  Drain: psum -> sbuf fp16 cast; 1/den via K=1 ones-matmul broadcast of the
  fp16 den row + one wide DVE reciprocal_approx_fast; zT = out * recip.
  y = zT^T @ (Wo*gate) + bo -> DRAM f16 (host accumulates f32).
"""

from contextlib import ExitStack

import numpy as np

import concourse.bacc as bacc
import concourse.bass as bass
import concourse.tile as tile
from concourse import mybir

P = 128
N = 2048          # tokens (one batch element per core)
D = 512           # model dim
CH = 256          # this core's qkv channel slice (4 heads x 64)
NKT = N // P      # 16 token tiles
QC = 512          # q-chunk width
NQC = N // QC     # 4
NC = 8            # chunks = (pr, qc)
LAG = 2           # attnv trails scores by this many kt-steps
F16 = mybir.dt.float16
F32 = mybir.dt.float32
AF = mybir.ActivationFunctionType
ALU = mybir.AluOpType

GELU_C = 0.7978845608028654   # sqrt(2/pi)
GELU_A = 0.044715


def build_program() -> bass.Bass:
    nc = bacc.Bacc()

    x_d = nc.declare_dram_parameter("x", [N, D], F16, isOutput=False)
    # packed: Wq | Wk | Wv | Ws2 along columns
    w4_d = nc.declare_dram_parameter("w4", [D, 4 * CH], F16, isOutput=False)
    wo_d = nc.declare_dram_parameter("wo", [CH, D], F16, isOutput=False)
    ws1_d = nc.declare_dram_parameter("ws1", [D, D], F16, isOutput=False)
    st_d = nc.declare_dram_parameter("style", [D], F16, isOutput=False)
    # packed f32 smalls: bq | bk | bs1 | bs2 | bo  (256+256+512+256+512)
    sm_d = nc.declare_dram_parameter("sm", [1792], F32, isOutput=False)
    bv_d = nc.declare_dram_parameter("bv", [CH], F32, isOutput=False)
    out_d = nc.declare_dram_parameter("out", [N, D], F16, isOutput=True)

    with ExitStack() as ctx:
        tc = ctx.enter_context(tile.TileContext(nc))
        const = ctx.enter_context(tc.tile_pool(name="const", bufs=1))

        # ---- warmup activation first on the ACT queue: pulls the single
        # exp_and_others table load to t~0, overlapping the input DMAs ----
        warm_in = const.tile([1, 2], F32)
        nc.vector.memset(warm_in, 0.0)
        warm_out = const.tile([1, 2], F16)
        nc.scalar.activation(warm_out, warm_in, AF.Exp)

        # ---- input DMAs (few, fat) ----
        xTs = [const.tile([P, N], F16, name=f"xT{i}") for i in range(4)]
        nc.sync.dma_start_transpose(xTs[0], x_d[:, 0:P])
        nc.scalar.dma_start_transpose(xTs[1], x_d[:, P : 2 * P])
        nc.sync.dma_start_transpose(xTs[2], x_d[:, 2 * P : 3 * P])
        nc.scalar.dma_start_transpose(xTs[3], x_d[:, 3 * P : 4 * P])
        w4 = const.tile([P, 4, 4 * CH], F16)
        nc.gpsimd.dma_start(w4, w4_d.rearrange("(k p) m -> p k m", p=P))
        wq = w4[:, :, 0:CH]
        wk = w4[:, :, CH : 2 * CH]
        wv = w4[:, :, 2 * CH : 3 * CH]
        ws2 = w4[:, :, 3 * CH : 4 * CH]
        smT = const.tile([P, 14], F32)
        nc.gpsimd.dma_start(smT, sm_d.rearrange("(c p) -> p c", p=P))
        bqT = smT[:, 0:2]
        bkT = smT[:, 2:4]
        bs1T = smT[:, 4:8]
        bs2T = smT[:, 8:10]
        ws1 = const.tile([P, 4, D], F16)
        nc.gpsimd.dma_start(ws1, ws1_d.rearrange("(k p) m -> p k m", p=P))
        stT = const.tile([P, 4], F16)
        nc.gpsimd.dma_start(stT, st_d.rearrange("(c p) -> p c", p=P))
        bvb = const.tile([P, CH], F32)
        nc.sync.dma_start(bvb, bv_d.rearrange("(o c) -> o c", o=1).to_broadcast((P, CH)))
        wo = const.tile([P, 2, D], F16)
        nc.gpsimd.dma_start(wo, wo_d.rearrange("(k p) m -> p k m", p=P))
        bob = const.tile([P, D], F32)
        nc.sync.dma_start(
            bob,
            sm_d[1280:1792].rearrange("(o c) -> o c", o=1).to_broadcast((P, D)),
        )

        ones_row = const.tile([1, 64], F16)
        nc.vector.memset(ones_row, 1.0)

        qT = const.tile([P, 2, N], F16)
        kT = const.tile([P, 2, N], F16)
        vv = const.tile([P, NKT, 4, 65], F16)  # per head: 64 v cols + ones col
        nc.vector.memset(vv[:, :, :, 64:65], 1.0)
        zT = const.tile([P, 2, N], F16)
        gate = const.tile([P, 2], F32)
        wog = const.tile([P, 2, D], F16)       # Wo * gate (rows scaled)

        with (
            tc.tile_pool(name="scp", bufs=2, space="PSUM") as scp,
            tc.tile_pool(name="outp", bufs=2, space="PSUM") as outp,
            tc.tile_pool(name="mps", bufs=2, space="PSUM") as mps,
            tc.tile_pool(name="eep", bufs=8) as eep,
            tc.tile_pool(name="osb", bufs=4) as osb,
            tc.tile_pool(name="dra", bufs=4) as dra,
            tc.tile_pool(name="ysb", bufs=3) as ysb,
        ):
            def emit_qk(dst, w, bias, m, qc):
                s_ = slice(qc * QC, (qc + 1) * QC)
                ps = mps.tile([P, QC], F32, tag="m")
                for k in range(4):
                    nc.tensor.matmul(
                        ps,
                        lhsT=w[:, k, m * P : (m + 1) * P],
                        rhs=xTs[k][:, s_],
                        start=(k == 0),
                        stop=(k == 3),
                    )
                nc.vector.tensor_scalar_add(dst[:, m, s_], ps, bias[:, m : m + 1])

            def emit_v(tt):
                psv = mps.tile([P, CH], F32, tag="m")
                for k in range(4):
                    nc.tensor.matmul(
                        psv,
                        lhsT=xTs[k][:, tt * P : (tt + 1) * P],
                        rhs=wv[:, k, :],
                        start=(k == 0),
                        stop=(k == 3),
                    )
                nc.vector.tensor_add(
                    vv[:, tt, :, 0:64],
                    psv.rearrange("p (h w) -> p h w", w=64),
                    bvb.rearrange("p (h w) -> p h w", w=64),
                )

            style_xg = [None]

            def emit_style_mm():
                hps = mps.tile([P, 4], F32, tag="m")
                for m in range(4):
                    for k in range(4):
                        nc.tensor.matmul(
                            hps[:, m : m + 1],
                            lhsT=ws1[:, k, m * P : (m + 1) * P],
                            rhs=stT[:, k : k + 1],
                            start=(k == 0),
                            stop=(k == 3),
                        )
                xg = dra.tile([P, 4], F32, tag="stx")
                nc.vector.tensor_add(xg, hps, bs1T)
                style_xg[0] = xg

            def emit_style_act():
                # gelu(x) ~= 0.5*x*(1+tanh(c*(x+a*x^3)));  sigmoid via tanh
                xg = style_xg[0]
                x2 = dra.tile([P, 4], F32, tag="st")
                nc.vector.tensor_mul(x2, xg, xg)
                x3 = dra.tile([P, 4], F32, tag="st")
                nc.vector.tensor_mul(x3, x2, xg)
                u0 = dra.tile([P, 4], F32, tag="st")
                nc.vector.tensor_scalar_mul(u0, x3, GELU_A)
                u = dra.tile([P, 4], F32, tag="st")
                nc.vector.tensor_add(u, u0, xg)
                th = dra.tile([P, 4], F32, tag="st")
                nc.scalar.activation(th, u, AF.Tanh, scale=GELU_C)
                g1 = dra.tile([P, 4], F32, tag="st")
                nc.vector.tensor_mul(g1, xg, th)
                g2 = dra.tile([P, 4], F32, tag="st")
                nc.vector.tensor_add(g2, g1, xg)
                hT = dra.tile([P, 4], F16, tag="st16")
                nc.vector.tensor_scalar_mul(hT, g2, 0.5)
                g_ps = mps.tile([P, 2], F32, tag="m")
                for m in range(2):
                    for k in range(4):
                        nc.tensor.matmul(
                            g_ps[:, m : m + 1],
                            lhsT=ws2[:, k, m * P : (m + 1) * P],
                            rhs=hT[:, k : k + 1],
                            start=(k == 0),
                            stop=(k == 3),
                        )
                pre = dra.tile([P, 2], F32, tag="st")
                nc.vector.tensor_add(pre, g_ps, bs2T)
                th2 = dra.tile([P, 2], F32, tag="st")
                nc.scalar.activation(th2, pre, AF.Tanh, scale=0.5)
                nc.vector.tensor_scalar(gate, th2, 0.5, 0.5, op0=ALU.mult, op1=ALU.add)
                for m in range(2):
                    nc.vector.tensor_scalar_mul(
                        wog[:, m, :], wo[:, m, :], gate[:, m : m + 1]
                    )

            def emit_y_tt(tt):
                ps = mps.tile([P, D], F32, tag="m")
                for m in range(2):
                    nc.tensor.matmul(
                        ps,
                        lhsT=zT[:, m, tt * P : (tt + 1) * P],
                        rhs=wog[:, m, :],
                        start=(m == 0),
                        stop=(m == 1),
                    )
                y = ysb.tile([P, D], F16, tag="ys")
                with nc.allow_low_precision(reason="fp16 output vs f32 ref"):
                    nc.vector.tensor_add(y, ps, bob)
                eng = nc.sync if tt % 2 == 0 else nc.gpsimd
                eng.dma_start(out_d[tt * P : (tt + 1) * P, :], y)

            # ---- pre-loop: minimum to unlock scores of chunk (pr0, qc0) ----
            emit_qk(kT, wk, bkT, 0, 0)
            emit_qk(qT, wq, bqT, 0, 0)

            # ---- filler schedule keyed by global kt-step ----
            fill = {}

            def add(k, fn):
                fill.setdefault(k, []).append(fn)

            add(0, lambda: emit_qk(kT, wk, bkT, 0, 1))
            add(5, lambda: emit_qk(kT, wk, bkT, 0, 2))
            add(9, lambda: emit_qk(kT, wk, bkT, 0, 3))
            for tt in range(NKT):           # v(tt) before attnv(tt) at k=tt+2
                add(tt + 1, lambda tt=tt: emit_v(tt))
            add(13, emit_style_mm)
            add(15, lambda: emit_qk(qT, wq, bqT, 0, 1))
            add(19, emit_style_act)
            add(18, lambda: emit_qk(kT, wk, bkT, 1, 0))
            add(24, lambda: emit_qk(kT, wk, bkT, 1, 1))
            add(20, lambda: emit_qk(qT, wq, bqT, 0, 2))
            add(30, lambda: emit_qk(kT, wk, bkT, 1, 2))
            add(36, lambda: emit_qk(kT, wk, bkT, 1, 3))
            add(40, lambda: emit_qk(qT, wq, bqT, 0, 3))
            add(52, lambda: emit_qk(qT, wq, bqT, 1, 0))
            add(68, lambda: emit_qk(qT, wq, bqT, 1, 1))
            add(84, lambda: emit_qk(qT, wq, bqT, 1, 2))
            add(100, lambda: emit_qk(qT, wq, bqT, 1, 3))
            # y(qc): zT for (1,qc) ready after drainB of chunk 4+qc (k=16c+19)
            for qc in range(3):
                for tt in range(4):
                    add(16 * (5 + qc) + 8 + 2 * tt,
                        lambda tt=tt, qc=qc: emit_y_tt(4 * qc + tt))

            chunks = [(pr, qc) for pr in range(2) for qc in range(NQC)]
            outs = {}    # c -> (out_lo, out_hi) psum
            outsb = {}   # c -> (sb_lo, sb_hi) sbuf f16 copies

            def drain_cast(c):
                # out psum stays until drain_norm (scheduler hoists the rcb
                # matmuls ahead of the waiting attnv, so no deadlock)
                outsb[c] = outs.pop(c)

            def drain_norm(c, final=False):
                # 1/den broadcast + normalize (v1-proven op shapes: custom-DVE
                # reciprocal must NOT have a partition-shifted input)
                pr, qc = chunks[c]
                s = slice(qc * QC, (qc + 1) * QC)
                out_lo, out_hi = outsb.pop(c)
                d16l = dra.tile([1, QC], F16, tag="r16")
                d16h = dra.tile([1, QC], F16, tag="r16")
                with nc.allow_low_precision(reason="fp16 denom O(1e3)"):
                    nc.vector.tensor_copy(d16l, out_lo[64:65, :])
                    nc.vector.tensor_copy(d16h, out_hi[64:65, :])
                rcb = mps.tile([P, QC], F32, tag="m")
                nc.tensor.matmul(
                    rcb[0:64, :], lhsT=ones_row, rhs=d16l,
                    start=True, stop=True, tile_position=(0, 0),
                )
                nc.tensor.matmul(
                    rcb[64:128, :], lhsT=ones_row, rhs=d16h,
                    start=True, stop=True, tile_position=(0, 64),
                )
                rcb32 = dra.tile([P, QC], F32, tag="rw")
                nc.vector.reciprocal_approx_fast(out=rcb32, in_=rcb)
                if final:
                    # 128-col pieces: each token tile's out-proj starts as
                    # soon as its zT slice is ready (shortens the tail)
                    for t4 in range(4):
                        cs = slice(t4 * P, (t4 + 1) * P)
                        so = slice(qc * QC + t4 * P, qc * QC + (t4 + 1) * P)
                        nc.vector.tensor_mul(
                            zT[0:64, pr, so], out_lo[0:64, cs], rcb32[0:64, cs]
                        )
                        nc.vector.tensor_mul(
                            zT[64:128, pr, so], out_hi[0:64, cs], rcb32[64:128, cs]
                        )
                        emit_y_tt(qc * 4 + t4)
                else:
                    nc.vector.tensor_mul(
                        zT[0:64, pr, s], out_lo[0:64, :], rcb32[0:64, :]
                    )
                    nc.vector.tensor_mul(
                        zT[64:128, pr, s], out_hi[0:64, :], rcb32[64:128, :]
                    )

            # ---- flat pipeline over 128 kt-steps (+LAG attnv slots) ----
            attnv_args = []
            for k in range(NKT * NC + LAG):
                if k < NKT * NC:
                    c, kt = divmod(k, NKT)
                    pr, qc = chunks[c]
                    if kt == 0:
                        outs[c] = (
                            outp.tile([65, QC], F32, tag="o", name=f"out_lo{c}"),
                            outp.tile([65, QC], F32, tag="o", name=f"out_hi{c}"),
                        )
                    s = slice(qc * QC, (qc + 1) * QC)
                    ks = slice(kt * P, (kt + 1) * P)
                    sc = scp.tile([P, 2, QC], F32, tag="sc")
                    nc.tensor.matmul(
                        sc[:, 0, :], lhsT=kT[0:64, pr, ks], rhs=qT[0:64, pr, s],
                        start=True, stop=True,
                    )
                    nc.tensor.matmul(
                        sc[:, 1, :], lhsT=kT[64:128, pr, ks], rhs=qT[64:128, pr, s],
                        start=True, stop=True,
                    )
                    ee = eep.tile([P, 2, QC], F16, tag="e")
                    nc.scalar.activation(ee, sc, AF.Exp)
                    attnv_args.append((ee[:, 0, :], ee[:, 1, :]))
                # previous chunk's cast-drain goes before this chunk's first
                # attnv (frees the out psum slots with just 2 DVE casts)
                j = k - LAG
                if j >= 0:
                    cj, kj = divmod(j, NKT)
                    if kj == 0 and cj >= 1:
                        drain_cast(cj - 1)
                for fn in fill.get(k, ()):
                    fn()
                if j >= 0:
                    cj, kj = divmod(j, NKT)
                    prj, _ = chunks[cj]
                    av_lo, av_hi = attnv_args[j]
                    nc.tensor.matmul(
                        outs[cj][0],
                        lhsT=vv[:, kj, 2 * prj, :],
                        rhs=av_lo,
                        start=(kj == 0), stop=(kj == NKT - 1),
                    )
                    nc.tensor.matmul(
                        outs[cj][1],
                        lhsT=vv[:, kj, 2 * prj + 1, :],
                        rhs=av_hi,
                        start=(kj == 0), stop=(kj == NKT - 1),
                    )
                    if kj == 1 and cj >= 1:
                        drain_norm(cj - 1)

            drain_cast(NC - 1)
            drain_norm(NC - 1, final=True)

    nc.finalize()
    return nc


_NC_CACHE = None


def _get_program() -> bass.Bass:
    global _NC_CACHE
    if _NC_CACHE is None:
        _NC_CACHE = build_program()
    return _NC_CACHE


def make_in_maps(inputs: dict) -> list[dict]:
    f16 = np.float16
    f32 = np.float32
    scale = 1.0 / 8.0  # 1/sqrt(head_dim), folded into Wq/bq
    x, style = inputs["x"], inputs["style"]
    in_maps = []
    for c in range(8):
        b, g = divmod(c, 2)
        ch = slice(CH * g, CH * (g + 1))
        w4 = np.concatenate(
            [
                inputs["Wq"][:, ch] * scale,
                inputs["Wk"][:, ch],
                inputs["Wv"][:, ch],
                inputs["Ws2"][:, ch],
            ],
            axis=1,
        )
        bo = inputs["bo"] if g == 0 else np.zeros_like(inputs["bo"])
        sm = np.concatenate(
            [
                inputs["bq"][ch] * scale,
                inputs["bk"][ch],
                inputs["bs1"],
                inputs["bs2"][ch],
                bo,
            ]
        )
        in_maps.append(
            {
                "x": np.ascontiguousarray(x[b]).astype(f16),
                "style": np.ascontiguousarray(style[b]).astype(f16),
                "w4": np.ascontiguousarray(w4).astype(f16),
                "wo": np.ascontiguousarray(inputs["Wo"][ch, :]).astype(f16),
                "ws1": np.ascontiguousarray(inputs["Ws1"]).astype(f16),
                "sm": np.ascontiguousarray(sm).astype(f32),
                "bv": np.ascontiguousarray(inputs["bv"][ch]).astype(f32),
            }
        )
    return in_maps


def kernel(**inputs) -> np.ndarray:
    from concourse.bass_utils import run_bass_kernel_spmd

    in_maps = make_in_maps(inputs)
    res = run_bass_kernel_spmd(_get_program(), in_maps, list(range(8))).results
    y = np.stack(
        [
            res[2 * b]["out"].astype(np.float32)
            + res[2 * b + 1]["out"].astype(np.float32)
            for b in range(4)
        ]
    )
    return y.astype(np.float32)
